# revision 15
# baseline (speedup 1.0000x reference)
"""Trainium2 Bass kernel for nn_Attn (additive/Bahdanau-style attention).

Math (per batch b):
    Wh, We   = W[:, :D], W[:, D:]                       # [D,D] each
    energy   = tanh(enc @ We.T + hidden @ Wh.T + b)     # [S, D]
    scores   = energy @ v, masked to length, softmax    # [S]
    context  = scores @ enc                             # [D]

Sharding / packing: data-parallel over batch B=16 across 8 cores, but
length-aware.  Positions >= lengths[b] are masked out of the softmax, so
only ceil(len/512)*512 positions per batch ever matter.  The host sorts
batches by padded tile count and pairs longest-with-shortest so every
core gets the same number NT of 512-wide s-tiles (5 for the reference
lengths instead of 8 for the naive full-S split).  Each core's two
batches are packed back-to-back into one flat tile list; the batch
structure (tile ownership, per-position validity) is carried entirely by
host-prepared relayout inputs (replicated hidden columns, owner masks,
position indices), so one SPMD program serves all cores.

Device-side structure:
  - pass 1 computes energy^T tiles [e=128, s=512] with We^T-stationary
    matmuls in bf16 (full PE rate, half the DMA/SBUF of fp32), looped
    (group, ec, kc, tile) so each weight chunk loads once per group.
  - the tanh bias (hid @ Wh^T + b) is computed on-device as
    [e-partition, tile] via stationary-Wh^T matmuls (no DRAM bounce).
  - the v-dot accumulates on the DVE; a per-tile partition-reduce matmul
    yields scores in [128, flat-chunk] layout, so the masked softmax is
    a handful of 128-lane ops.  exp uses the static bound M = sum|v| >=
    max(score) (softmax shift-invariance; |tanh| <= 1) -- no max-reduce.
  - pass 2 accumulates BOTH batch contexts at once into one [2, D] PSUM
    group: the stationary operand is [s=128, 2] of masked, batch-selected
    exp weights.  Normalization by 1/sum folds into the output scale.
  - tiles are processed in groups ([0], [1,2], [3,4], ...): the first
    group starts compute after a minimal DMA prefix, later groups reuse
    each weight load across member tiles, and every group's softmax +
    pass-2 work is emitted interleaved into the next group's matmul
    stream so the PE never waits on the DVE chain.
"""

import numpy as np

B, S, D = 16, 2048, 1024
NCORES = 8
BL = B // NCORES   # batches per core
ST = 512           # s-tile width (pass-1 moving dim; one PSUM bank)
DC = D // 128      # contraction / e chunks
NPT = ST // 128    # 128-wide flat chunks per s-tile

_NC_CACHE = {}


def _build_program(nt, stage="all"):
    import concourse.bacc as bacc
    import concourse.bass as bass
    import concourse.mybir as mybir
    import concourse.tile as tile

    f32 = mybir.dt.float32
    bf16 = mybir.dt.bfloat16
    Tanh = mybir.ActivationFunctionType.Tanh
    Exp = mybir.ActivationFunctionType.Exp
    Identity = mybir.ActivationFunctionType.Identity
    Alu = mybir.AluOpType

    nf = nt * NPT        # flat 128-wide chunks per core
    d = D

    # tile groups: [0] alone (fast start after a small DMA prefix), then
    # pairs, with a singleton LAST group so the tail dependency chain
    # (reduce -> exp -> attn2 -> pass-2) covers only one tile.
    groups = [[0]]
    rem = list(range(1, nt))
    while rem:
        k = 2 if len(rem) > 2 else 1
        groups.append(rem[:k])
        rem = rem[k:]

    nc = bacc.Bacc()
    # all big inputs are host-prearranged partition-major so every DMA is a
    # straight [128, X] copy with one contiguous line per partition.  The
    # weight matrices are further split into per-ec stripes so pass-1 can
    # start as soon as stripe 0 lands (~1 MiB of DMA instead of ~3 MiB).
    encTf_d = nc.declare_dram_parameter("encTf", [nt, 128, DC, ST], bf16, isOutput=False)
    encf_d = nc.declare_dram_parameter("encf", [nf, 128, d], bf16, isOutput=False)
    weTs_d = nc.declare_dram_parameter("weTs", [DC, 128, DC, 128], bf16, isOutput=False)
    whTs_d = nc.declare_dram_parameter("whTs", [DC, 128, DC, 128], bf16, isOutput=False)
    hidf_d = nc.declare_dram_parameter("hidf", [128, DC, nt], bf16, isOutput=False)
    bcol_d = nc.declare_dram_parameter("bcol", [128, DC], f32, isOutput=False)
    vcol_d = nc.declare_dram_parameter("vcol", [128, DC], f32, isOutput=False)
    posf_d = nc.declare_dram_parameter("posf", [128, nf], f32, isOutput=False)
    lenmap_d = nc.declare_dram_parameter("lenmap", [128, nf], f32, isOutput=False)
    own0_d = nc.declare_dram_parameter("own0", [128, nf], f32, isOutput=False)
    if stage == "all":
        out_d = nc.declare_dram_parameter("ctx_out", [BL, d], f32, isOutput=True)
    else:
        out_d = nc.declare_dram_parameter("ctx_out", [128, nf], f32, isOutput=True)

    with tile.TileContext(nc) as tc:
        with (
            tc.tile_pool(name="consts", bufs=1) as consts,
            tc.tile_pool(name="etp", bufs=nt) as etp,
            tc.tile_pool(name="enf", bufs=nf) as enf,
            tc.tile_pool(name="enp", bufs=4) as enp,
            tc.tile_pool(name="psA", bufs=4, space="PSUM") as psA,
            tc.tile_pool(name="psS", bufs=2, space="PSUM") as psS,
            tc.tile_pool(name="psM", bufs=1, space="PSUM") as psM,
        ):
            # ---------------- DMA emission ---------------------------------
            # tiny consts on the (otherwise idle) gpsimd queue; the ordered
            # big-tensor stream on the sync queue: encTf[0] first, then
            # alternating whT/weT ec-stripes (consumed in that order by the
            # merged hid+pass-1 ec loop), then the rest.
            vcol_sb = consts.tile([128, DC], f32)
            nc.gpsimd.dma_start(out=vcol_sb, in_=vcol_d[:, :])
            bcol_sb = consts.tile([128, DC], f32)
            nc.gpsimd.dma_start(out=bcol_sb, in_=bcol_d[:, :])
            hidf_sb = consts.tile([128, DC, nt], bf16)
            nc.gpsimd.dma_start(out=hidf_sb, in_=hidf_d[:, :, :])
            posf_sb = consts.tile([128, nf], f32)
            nc.gpsimd.dma_start(out=posf_sb, in_=posf_d[:, :])
            lenmap_sb = consts.tile([128, nf], f32)
            nc.gpsimd.dma_start(out=lenmap_sb, in_=lenmap_d[:, :])
            own0_sb = consts.tile([128, nf], f32)
            nc.gpsimd.dma_start(out=own0_sb, in_=own0_d[:, :])
            et_tiles = [etp.tile([128, DC, ST], bf16, tag="et", name="et0")]
            nc.sync.dma_start(out=et_tiles[0], in_=encTf_d[0])
            whT_tiles = []
            weT_tiles = []
            for ec in range(DC):
                wh = consts.tile([128, DC, 128], bf16, name=f"whT{ec}")
                nc.sync.dma_start(out=wh, in_=whTs_d[ec])
                whT_tiles.append(wh)
                we = consts.tile([128, DC, 128], bf16, name=f"weT{ec}")
                nc.sync.dma_start(out=we, in_=weTs_d[ec])
                weT_tiles.append(we)
            for t_ in range(1, nt):
                et = etp.tile([128, DC, ST], bf16, tag="et", name=f"et{t_}")
                nc.sync.dma_start(out=et, in_=encTf_d[t_])
                et_tiles.append(et)
            en2_tiles = []
            for f in range(nf):
                en2 = enf.tile([128, d], bf16, tag="en2")
                nc.sync.dma_start(out=en2, in_=encf_d[f])
                en2_tiles.append(en2)

            # ---------------- small constants ----------------------------
            ones_sb = consts.tile([128, 1], f32)
            nc.vector.memset(ones_sb, 1.0)
            ones_row = consts.tile([1, 128], f32)
            nc.vector.memset(ones_row, 1.0)
            # Upper bound M = sum|v| >= any score (|tanh|<=1): replaces the
            # serial max-reduce in the softmax.
            vabs = consts.tile([128, 1], f32)
            nc.vector.reduce_sum(
                out=vabs, in_=vcol_sb, axis=mybir.AxisListType.X,
                apply_absolute_value=True,
            )
            psv = psS.tile([1, 1], f32, tag="s", name="psv")
            nc.tensor.matmul(psv, ones_sb[:, 0:1], vabs, start=True, stop=True)
            mtot = consts.tile([1, 1], f32)
            nc.vector.tensor_copy(mtot, psv)
            psb = psS.tile([128, 1], f32, tag="s", name="psb")
            nc.tensor.matmul(psb, ones_row[:, :], mtot[:, :], start=True, stop=True)
            negM = consts.tile([128, 1], f32)
            nc.scalar.mul(negM, psb, -1.0)

            # masks from host-relayout index tensors: valid = pos < len,
            # then split by batch-slot ownership.
            valid_sb = consts.tile([128, nf], f32)
            nc.vector.scalar_tensor_tensor(
                valid_sb, posf_sb, 1.0, lenmap_sb, op0=Alu.mult, op1=Alu.is_lt
            )
            mask0 = consts.tile([128, nf], f32)
            nc.vector.scalar_tensor_tensor(
                mask0, valid_sb, 1.0, own0_sb, op0=Alu.mult, op1=Alu.mult
            )
            mask1 = consts.tile([128, nf], f32)
            nc.vector.scalar_tensor_tensor(
                mask1, valid_sb, 1.0, mask0, op0=Alu.mult, op1=Alu.subtract
            )

            # ---------------- pass 1 + pipelined softmax / pass 2 ---------
            # The hid-bias matmuls ((hid @ Wh^T + b)^T via stationary-Wh^T,
            # [128e, nt] per ec) are folded into group 0's ec loop so each
            # iteration consumes exactly the whT/weT stripe pair the DMA
            # stream delivers next.
            bias_all = consts.tile([128, DC, nt], f32)
            scores_sb = consts.tile([128, nf], f32)
            exp_sb = consts.tile([128, nf], f32)
            attn2b = consts.tile([128, nf, 2], bf16)
            mexp0 = consts.tile([128, nf], f32)
            mexp1 = consts.tile([128, nf], f32)
            psums01 = consts.tile([128, 2], f32)
            cps = psM.tile([BL, d], f32, tag="m", name="cps")

            pend = None            # (tiles, accs) of the previous group
            p2_emitted = 0         # flat chunks whose pass-2 mm is emitted

            def emit_reduces(tiles, accs):
                # partition-reduce each acc column block into scores_sb.
                # All chunks of the pending group go into one PSUM tile
                # (separate cols) so nothing serializes on ring reuse.
                sps = psS.tile([128, NPT * len(tiles)], f32, tag="s")
                for j, t_ in enumerate(tiles):
                    for c in range(NPT):
                        nc.tensor.matmul(
                            sps[:, j * NPT + c:j * NPT + c + 1],
                            accs[t_][:, c * 128:(c + 1) * 128],
                            ones_sb[:, 0:1],
                            start=True,
                            stop=True,
                        )
                f0 = tiles[0] * NPT
                f1 = tiles[-1] * NPT + NPT
                nc.vector.tensor_copy(scores_sb[:, f0:f1], sps)

            def emit_softmax(tiles):
                f0 = tiles[0] * NPT
                f1 = tiles[-1] * NPT + NPT
                nc.scalar.activation(
                    exp_sb[:, f0:f1], scores_sb[:, f0:f1], Exp, bias=negM[:, 0:1]
                )
                nc.vector.scalar_tensor_tensor(
                    attn2b[:, f0:f1, 0], exp_sb[:, f0:f1], 1.0, mask0[:, f0:f1],
                    op0=Alu.mult, op1=Alu.mult,
                )
                nc.vector.scalar_tensor_tensor(
                    attn2b[:, f0:f1, 1], exp_sb[:, f0:f1], 1.0, mask1[:, f0:f1],
                    op0=Alu.mult, op1=Alu.mult,
                )

            def emit_pass2(tiles):
                nonlocal p2_emitted
                f0 = tiles[0] * NPT
                f1 = tiles[-1] * NPT + NPT
                for f in range(f0, f1):
                    for h in range(2):
                        nc.tensor.matmul(
                            cps[:, h * 512:(h + 1) * 512],
                            attn2b[:, f, :],
                            en2_tiles[f][:, h * 512:(h + 1) * 512],
                            start=(f == 0),
                            stop=(f == nf - 1),
                        )
                p2_emitted = f1

            for gi, tiles in enumerate(groups):
                accs = {}
                for ec in range(DC):
                    if gi == 0:
                        psh = psS.tile([128, nt], f32, tag="s", name=f"psh{ec}")
                        for kc in range(DC):
                            nc.tensor.matmul(
                                psh,
                                whT_tiles[ec][:, kc, :],
                                hidf_sb[:, kc, :],
                                start=(kc == 0),
                                stop=(kc == DC - 1),
                            )
                    pss = {
                        t_: psA.tile([128, ST], f32, tag="proj", name=f"ps{t_}_{ec}")
                        for t_ in tiles
                    }
                    for kc in range(DC):
                        for t_ in tiles:
                            nc.tensor.matmul(
                                pss[t_],
                                weT_tiles[ec][:, kc, :],
                                et_tiles[t_][:, kc, :],
                                start=(kc == 0),
                                stop=(kc == DC - 1),
                            )
                    if gi == 0:
                        nc.scalar.activation(
                            bias_all[:, ec, :], psh, Identity,
                            bias=bcol_sb[:, ec:ec + 1],
                        )
                    # deferred post-work of the previous group, staged so the
                    # PE queue always has matmul runway ahead of the deps.
                    if pend is not None:
                        if ec == 1:
                            emit_reduces(*pend)
                        elif ec == 3:
                            emit_softmax(pend[0])
                        elif ec == 5:
                            emit_pass2(pend[0])
                            pend = None
                    for t_ in tiles:
                        en = enp.tile([128, ST], f32, tag="en")
                        nc.scalar.activation(
                            en, pss[t_], Tanh, bias=bias_all[:, ec, t_:t_ + 1]
                        )
                        if ec == 0:
                            acc = enp.tile([128, ST], f32, tag="acc", bufs=5)
                            accs[t_] = acc
                            nc.vector.tensor_scalar_mul(acc, en, vcol_sb[:, 0:1])
                        else:
                            nc.vector.scalar_tensor_tensor(
                                accs[t_], en, vcol_sb[:, ec:ec + 1], accs[t_],
                                op0=Alu.mult, op1=Alu.add,
                            )
                pend = (tiles, accs)

            # tail: post-work of the last group
            emit_reduces(*pend)
            emit_softmax(pend[0])
            if stage == "p1":
                nc.gpsimd.dma_start(out=out_d[:, :], in_=scores_sb)
            elif stage == "sm":
                nc.gpsimd.dma_start(out=out_d[:, :], in_=exp_sb)
            else:
                # softmax denominators -> 1/sum, BEFORE the last pass-2 mms
                # so only the final scale remains on the tail chain.
                nc.vector.scalar_tensor_tensor(
                    mexp0, exp_sb, 1.0, mask0, op0=Alu.mult, op1=Alu.mult,
                    accum_out=psums01[:, 0:1],
                )
                nc.vector.scalar_tensor_tensor(
                    mexp1, exp_sb, 1.0, mask1, op0=Alu.mult, op1=Alu.mult,
                    accum_out=psums01[:, 1:2],
                )
                pst = psS.tile([BL, 1], f32, tag="s", name="pst")
                nc.tensor.matmul(pst, psums01, ones_sb[:, 0:1], start=True, stop=True)
                rinv2 = consts.tile([BL, 1], f32)
                nc.vector.reciprocal(rinv2, pst)
                emit_pass2(pend[0])
                assert p2_emitted == nf
                ctx_sb = consts.tile([BL, d], f32)
                nc.vector.tensor_scalar_mul(ctx_sb, cps, rinv2)
                nc.gpsimd.dma_start(out=out_d[:, :], in_=ctx_sb)

    nc.compile()
    return nc


def _get_nc(nt, stage="all"):
    key = (nt, stage)
    if key not in _NC_CACHE:
        _NC_CACHE[key] = _build_program(nt, stage)
    return _NC_CACHE[key]


def _plan(lengths):
    """Pair batches (longest padded length with shortest) so every core's
    two batches need the same, minimal number of 512-wide s-tiles."""
    l = np.asarray(lengths, dtype=np.int64)
    c = (np.clip(l, 1, S) + ST - 1) // ST          # tiles per batch, >= 1
    order = np.argsort(-c, kind="stable")
    pairs = [(int(order[i]), int(order[B - 1 - i])) for i in range(NCORES)]
    nt = int(max(c[a] + c[b] for a, b in pairs))
    return pairs, c, nt


def _make_in_maps(encoder_outputs, hidden, lengths, W, b, v):
    import ml_dtypes

    bf16 = ml_dtypes.bfloat16
    enc = np.asarray(encoder_outputs, dtype=np.float32)
    hid = np.asarray(hidden, dtype=np.float32)
    len_ = np.asarray(lengths, dtype=np.int64)
    Wn = np.asarray(W, dtype=np.float32)
    bn = np.asarray(b, dtype=np.float32)
    vn = np.asarray(v, dtype=np.float32)

    pairs, c, nt = _plan(len_)
    nf = nt * NPT

    # per-ec stripes, partition-major within each stripe:
    # [ec, p, kc, q] = W.T[kc*128 + p, ec*128 + q]
    weTs = np.ascontiguousarray(
        Wn[:, D:].T.reshape(DC, 128, DC, 128).transpose(2, 1, 0, 3).astype(bf16)
    )
    whTs = np.ascontiguousarray(
        Wn[:, :D].T.reshape(DC, 128, DC, 128).transpose(2, 1, 0, 3).astype(bf16)
    )
    bcol = np.ascontiguousarray(bn.reshape(DC, 128).T)
    vcol = np.ascontiguousarray(vn.reshape(DC, 128).T)

    in_maps = []
    for a, b_ in pairs:
        na, nb = int(c[a]), int(c[b_])
        packed = np.zeros((nt * ST, D), dtype=np.float32)
        packed[:na * ST] = enc[a, :na * ST]
        packed[na * ST:(na + nb) * ST] = enc[b_, :nb * ST]
        packed = packed.astype(bf16)
        encTf = np.ascontiguousarray(
            packed.reshape(nt, ST, DC, 128).transpose(0, 3, 2, 1)
        )
        encf = packed.reshape(nf, 128, D)

        hidf = np.zeros((D, nt), dtype=np.float32)
        hidf[:, :na] = hid[a][:, None]
        hidf[:, na:na + nb] = hid[b_][:, None]
        hidf = np.ascontiguousarray(
            hidf.astype(bf16).reshape(DC, 128, nt).transpose(1, 0, 2)
        )

        posf = np.full((128, nf), 1.0e9, dtype=np.float32)
        lenmap = np.zeros((128, nf), dtype=np.float32)
        own0 = np.zeros((128, nf), dtype=np.float32)
        p = np.arange(128, dtype=np.float32)
        for f in range(nf):
            t = f // NPT
            if t < na:
                posf[:, f] = f * 128 + p
                lenmap[:, f] = float(len_[a])
                own0[:, f] = 1.0
            elif t < na + nb:
                posf[:, f] = (f - na * NPT) * 128 + p
                lenmap[:, f] = float(len_[b_])

        in_maps.append(
            dict(
                encTf=encTf, encf=np.ascontiguousarray(encf),
                weTs=weTs, whTs=whTs, hidf=hidf,
                bcol=bcol, vcol=vcol,
                posf=posf, lenmap=lenmap, own0=own0,
            )
        )
    return in_maps, pairs, nt


def run(inputs, trace=False, stage="all"):
    """Run on 8 NeuronCores; returns (output [B,1,D], BassKernelResults)."""
    from concourse.bass_utils import run_bass_kernel_spmd

    in_maps, pairs, nt = _make_in_maps(**inputs)
    nc = _get_nc(nt, stage)
    r = run_bass_kernel_spmd(
        nc, in_maps, core_ids=list(range(NCORES)), trace=trace
    )
    if stage != "all":
        out = np.stack(
            [np.asarray(r.results[i]["ctx_out"]) for i in range(NCORES)], axis=0
        )
        return out, r, pairs
    out = np.empty((B, 1, D), dtype=np.float32)
    for i, (a, b_) in enumerate(pairs):
        ctx = np.asarray(r.results[i]["ctx_out"])
        out[a, 0] = ctx[0]
        out[b_, 0] = ctx[1]
    return out, r


def kernel(encoder_outputs, hidden, lengths, W, b, v):
    out, _ = run(
        dict(
            encoder_outputs=encoder_outputs,
            hidden=hidden,
            lengths=lengths,
            W=W,
            b=b,
            v=v,
        )
    )
    return out


# revision 17
# speedup vs baseline: 1.2084x; 1.2084x over previous
"""Trainium2 Bass kernel for nn_Attn (additive/Bahdanau-style attention).

Math (per batch b):
    Wh, We   = W[:, :D], W[:, D:]                       # [D,D] each
    energy   = tanh(enc @ We.T + hidden @ Wh.T + b)     # [S, D]
    scores   = energy @ v, masked to length, softmax    # [S]
    context  = scores @ enc                             # [D]

Sharding / packing: data-parallel over batch B=16 across 8 cores, but
length-aware.  Positions >= lengths[b] are masked out of the softmax, so
only ceil(len/512)*512 positions per batch ever matter.  The host sorts
batches by padded tile count and pairs longest-with-shortest so every
core gets the same number NT of 512-wide s-tiles (5 for the reference
lengths instead of 8 for the naive full-S split).  Each core's two
batches are packed back-to-back into one flat tile list; the batch
structure (tile ownership, per-position validity) is carried entirely by
host-prepared relayout inputs (replicated hidden columns, owner masks,
position indices), so one SPMD program serves all cores.

Device-side structure:
  - pass 1 computes energy^T tiles [e=128, s=512] with We^T-stationary
    matmuls in bf16 (full PE rate, half the DMA/SBUF of fp32), looped
    (group, ec, kc, tile) so each weight chunk loads once per group.
  - the tanh bias (hid @ Wh^T + b) is computed on-device as
    [e-partition, tile] via stationary-Wh^T matmuls (no DRAM bounce).
  - the v-dot accumulates on the DVE; a per-tile partition-reduce matmul
    yields scores in [128, flat-chunk] layout, so the masked softmax is
    a handful of 128-lane ops.  exp uses the static bound M = sum|v| >=
    max(score) (softmax shift-invariance; |tanh| <= 1) -- no max-reduce.
  - pass 2 accumulates BOTH batch contexts at once into one [2, D] PSUM
    group: the stationary operand is [s=128, 2] of masked, batch-selected
    exp weights.  Normalization by 1/sum folds into the output scale.
  - tiles are processed in groups ([0], [1,2], [3,4], ...): the first
    group starts compute after a minimal DMA prefix, later groups reuse
    each weight load across member tiles, and every group's softmax +
    pass-2 work is emitted interleaved into the next group's matmul
    stream so the PE never waits on the DVE chain.
"""

import numpy as np

B, S, D = 16, 2048, 1024
NCORES = 8
BL = B // NCORES   # batches per core
ST = 512           # s-tile width (pass-1 moving dim; one PSUM bank)
DC = D // 128      # contraction / e chunks
NPT = ST // 128    # 128-wide flat chunks per s-tile

_NC_CACHE = {}


def _build_program(nt, stage="all"):
    import concourse.bacc as bacc
    import concourse.bass as bass
    import concourse.mybir as mybir
    import concourse.tile as tile

    f32 = mybir.dt.float32
    bf16 = mybir.dt.bfloat16
    Tanh = mybir.ActivationFunctionType.Tanh
    Exp = mybir.ActivationFunctionType.Exp
    Identity = mybir.ActivationFunctionType.Identity
    Alu = mybir.AluOpType

    nf = nt * NPT        # flat 128-wide chunks per core
    d = D

    # tile groups: [0] alone (fast start after a small DMA prefix), then
    # pairs, with a singleton LAST group so the tail dependency chain
    # (reduce -> exp -> attn2 -> pass-2) covers only one tile.
    groups = [[0]]
    rem = list(range(1, nt))
    while rem:
        k = 2 if len(rem) > 2 else 1
        groups.append(rem[:k])
        rem = rem[k:]

    nc = bacc.Bacc()
    # all big inputs are host-prearranged partition-major so every DMA is a
    # straight [128, X] copy with one contiguous line per partition.  The
    # weight matrices are further split into per-ec stripes so pass-1 can
    # start as soon as stripe 0 lands (~1 MiB of DMA instead of ~3 MiB).
    encTf_d = nc.declare_dram_parameter("encTf", [nt, 128, DC, ST], bf16, isOutput=False)
    encf_d = nc.declare_dram_parameter("encf", [nf, 128, d], bf16, isOutput=False)
    weTs_d = nc.declare_dram_parameter("weTs", [DC, 128, DC, 128], bf16, isOutput=False)
    whTs_d = nc.declare_dram_parameter("whTs", [DC, 128, DC, 128], bf16, isOutput=False)
    hidf_d = nc.declare_dram_parameter("hidf", [128, DC, nt], bf16, isOutput=False)
    bcol_d = nc.declare_dram_parameter("bcol", [128, DC], f32, isOutput=False)
    vcol_d = nc.declare_dram_parameter("vcol", [128, DC], f32, isOutput=False)
    posf_d = nc.declare_dram_parameter("posf", [128, nf], f32, isOutput=False)
    lenmap_d = nc.declare_dram_parameter("lenmap", [128, nf], f32, isOutput=False)
    own0_d = nc.declare_dram_parameter("own0", [128, nf], f32, isOutput=False)
    if stage == "all":
        out_d = nc.declare_dram_parameter("ctx_out", [BL, d], f32, isOutput=True)
    else:
        out_d = nc.declare_dram_parameter("ctx_out", [128, nf], f32, isOutput=True)

    with tile.TileContext(nc) as tc:
        with (
            tc.tile_pool(name="consts", bufs=1) as consts,
            tc.tile_pool(name="etp", bufs=nt) as etp,
            tc.tile_pool(name="enf", bufs=nf) as enf,
            tc.tile_pool(name="enp", bufs=4) as enp,
            tc.tile_pool(name="psA", bufs=4, space="PSUM") as psA,
            tc.tile_pool(name="psS", bufs=2, space="PSUM") as psS,
            tc.tile_pool(name="psM", bufs=1, space="PSUM") as psM,
        ):
            # ---------------- DMA emission ---------------------------------
            # tiny consts on the (otherwise idle) gpsimd queue; the ordered
            # big-tensor stream on the sync queue: encTf[0] first, then
            # alternating whT/weT ec-stripes (consumed in that order by the
            # merged hid+pass-1 ec loop), then the rest.
            vcol_sb = consts.tile([128, DC], f32)
            nc.gpsimd.dma_start(out=vcol_sb, in_=vcol_d[:, :])
            bcol_sb = consts.tile([128, DC], f32)
            nc.gpsimd.dma_start(out=bcol_sb, in_=bcol_d[:, :])
            hidf_sb = consts.tile([128, DC, nt], bf16)
            nc.gpsimd.dma_start(out=hidf_sb, in_=hidf_d[:, :, :])
            posf_sb = consts.tile([128, nf], f32)
            nc.gpsimd.dma_start(out=posf_sb, in_=posf_d[:, :])
            lenmap_sb = consts.tile([128, nf], f32)
            nc.gpsimd.dma_start(out=lenmap_sb, in_=lenmap_d[:, :])
            own0_sb = consts.tile([128, nf], f32)
            nc.gpsimd.dma_start(out=own0_sb, in_=own0_d[:, :])
            et_tiles = [etp.tile([128, DC, ST], bf16, tag="et", name="et0")]
            nc.sync.dma_start(out=et_tiles[0], in_=encTf_d[0])
            whTs_sb = consts.tile([128, DC, DC, 128], bf16, name="whTs")
            weTs_sb = consts.tile([128, DC, DC, 128], bf16, name="weTs")
            for ec in range(DC):
                nc.sync.dma_start(out=whTs_sb[:, ec], in_=whTs_d[ec])
                nc.sync.dma_start(out=weTs_sb[:, ec], in_=weTs_d[ec])
            for t_ in range(1, nt):
                et = etp.tile([128, DC, ST], bf16, tag="et", name=f"et{t_}")
                nc.sync.dma_start(out=et, in_=encTf_d[t_])
                et_tiles.append(et)
            en2_tiles = []
            for f in range(nf):
                en2 = enf.tile([128, d], bf16, tag="en2")
                nc.sync.dma_start(out=en2, in_=encf_d[f])
                en2_tiles.append(en2)

            # ---------------- small constants ----------------------------
            ones_sb = consts.tile([128, 1], f32)
            nc.vector.memset(ones_sb, 1.0)
            ones_row = consts.tile([1, 128], f32)
            nc.vector.memset(ones_row, 1.0)
            # Upper bound M = sum|v| >= any score (|tanh|<=1): replaces the
            # serial max-reduce in the softmax.
            vabs = consts.tile([128, 1], f32)
            nc.vector.reduce_sum(
                out=vabs, in_=vcol_sb, axis=mybir.AxisListType.X,
                apply_absolute_value=True,
            )
            psv = psS.tile([1, 1], f32, tag="s", name="psv")
            nc.tensor.matmul(psv, ones_sb[:, 0:1], vabs, start=True, stop=True)
            mtot = consts.tile([1, 1], f32)
            nc.vector.tensor_copy(mtot, psv)
            psb = psS.tile([128, 1], f32, tag="s", name="psb")
            nc.tensor.matmul(psb, ones_row[:, :], mtot[:, :], start=True, stop=True)
            negM = consts.tile([128, 1], f32)
            nc.scalar.mul(negM, psb, -1.0)

            # masks from host-relayout index tensors: valid = pos < len,
            # then split by batch-slot ownership.
            valid_sb = consts.tile([128, nf], f32)
            nc.vector.scalar_tensor_tensor(
                valid_sb, posf_sb, 1.0, lenmap_sb, op0=Alu.mult, op1=Alu.is_lt
            )
            mask0 = consts.tile([128, nf], f32)
            nc.vector.scalar_tensor_tensor(
                mask0, valid_sb, 1.0, own0_sb, op0=Alu.mult, op1=Alu.mult
            )
            mask1 = consts.tile([128, nf], f32)
            nc.vector.scalar_tensor_tensor(
                mask1, valid_sb, 1.0, mask0, op0=Alu.mult, op1=Alu.subtract
            )

            # ---------------- pass 1 + pipelined softmax / pass 2 ---------
            # The hid-bias matmuls ((hid @ Wh^T + b)^T via stationary-Wh^T,
            # [128e, nt] per ec) are folded into group 0's ec loop so each
            # iteration consumes exactly the whT/weT stripe pair the DMA
            # stream delivers next.
            bias_all = consts.tile([128, DC, nt], f32)
            scores_sb = consts.tile([128, nf], f32)
            exp_sb = consts.tile([128, nf], f32)
            attn2b = consts.tile([128, nf, 2], bf16)
            mexp0 = consts.tile([128, nf], f32)
            mexp1 = consts.tile([128, nf], f32)
            psums01 = consts.tile([128, 2], f32)
            cps = psM.tile([BL, d], f32, tag="m", name="cps")

            pend = None            # (tiles, accs) of the previous group
            p2_emitted = 0         # flat chunks whose pass-2 mm is emitted

            def emit_reduces(tiles, accs):
                # partition-reduce each acc column block into scores_sb.
                # All chunks of the pending group go into one PSUM tile
                # (separate cols) so nothing serializes on ring reuse.
                sps = psS.tile([128, NPT * len(tiles)], f32, tag="s")
                for j, t_ in enumerate(tiles):
                    for c in range(NPT):
                        nc.tensor.matmul(
                            sps[:, j * NPT + c:j * NPT + c + 1],
                            accs[t_][:, c * 128:(c + 1) * 128],
                            ones_sb[:, 0:1],
                            start=True,
                            stop=True,
                        )
                f0 = tiles[0] * NPT
                f1 = tiles[-1] * NPT + NPT
                nc.vector.tensor_copy(scores_sb[:, f0:f1], sps)

            def emit_softmax(tiles):
                f0 = tiles[0] * NPT
                f1 = tiles[-1] * NPT + NPT
                nc.scalar.activation(
                    exp_sb[:, f0:f1], scores_sb[:, f0:f1], Exp, bias=negM[:, 0:1]
                )
                nc.vector.scalar_tensor_tensor(
                    attn2b[:, f0:f1, 0], exp_sb[:, f0:f1], 1.0, mask0[:, f0:f1],
                    op0=Alu.mult, op1=Alu.mult,
                )
                nc.vector.scalar_tensor_tensor(
                    attn2b[:, f0:f1, 1], exp_sb[:, f0:f1], 1.0, mask1[:, f0:f1],
                    op0=Alu.mult, op1=Alu.mult,
                )

            def emit_pass2(tiles):
                nonlocal p2_emitted
                f0 = tiles[0] * NPT
                f1 = tiles[-1] * NPT + NPT
                for f in range(f0, f1):
                    for h in range(2):
                        nc.tensor.matmul(
                            cps[:, h * 512:(h + 1) * 512],
                            attn2b[:, f, :],
                            en2_tiles[f][:, h * 512:(h + 1) * 512],
                            start=(f == 0),
                            stop=(f == nf - 1),
                        )
                p2_emitted = f1

            for gi, tiles in enumerate(groups):
                accs = {}
                for ec in range(DC):
                    if gi == 0:
                        psh = psS.tile([128, nt], f32, tag="s", name=f"psh{ec}")
                        for kc in range(DC):
                            nc.tensor.matmul(
                                psh,
                                whTs_sb[:, ec, kc, :],
                                hidf_sb[:, kc, :],
                                start=(kc == 0),
                                stop=(kc == DC - 1),
                            )
                    pss = {
                        t_: psA.tile([128, ST], f32, tag="proj", name=f"ps{t_}_{ec}")
                        for t_ in tiles
                    }
                    for kc in range(DC):
                        for t_ in tiles:
                            nc.tensor.matmul(
                                pss[t_],
                                weTs_sb[:, ec, kc, :],
                                et_tiles[t_][:, kc, :],
                                start=(kc == 0),
                                stop=(kc == DC - 1),
                            )
                    if gi == 0:
                        nc.scalar.activation(
                            bias_all[:, ec, :], psh, Identity,
                            bias=bcol_sb[:, ec:ec + 1],
                        )
                    # deferred post-work of the previous group, staged so the
                    # PE queue always has matmul runway ahead of the deps.
                    if pend is not None:
                        if ec == 1:
                            emit_reduces(*pend)
                        elif ec == 3:
                            emit_softmax(pend[0])
                        elif ec == 5:
                            emit_pass2(pend[0])
                            pend = None
                    for t_ in tiles:
                        en = enp.tile([128, ST], f32, tag="en")
                        nc.scalar.activation(
                            en, pss[t_], Tanh, bias=bias_all[:, ec, t_:t_ + 1]
                        )
                        if ec == 0:
                            acc = enp.tile([128, ST], f32, tag="acc", bufs=5)
                            accs[t_] = acc
                            nc.vector.tensor_scalar_mul(acc, en, vcol_sb[:, 0:1])
                        else:
                            nc.vector.scalar_tensor_tensor(
                                accs[t_], en, vcol_sb[:, ec:ec + 1], accs[t_],
                                op0=Alu.mult, op1=Alu.add,
                            )
                pend = (tiles, accs)

            # tail: post-work of the last group
            emit_reduces(*pend)
            emit_softmax(pend[0])
            if stage == "p1":
                nc.gpsimd.dma_start(out=out_d[:, :], in_=scores_sb)
            elif stage == "sm":
                nc.gpsimd.dma_start(out=out_d[:, :], in_=exp_sb)
            else:
                # softmax denominators -> 1/sum, BEFORE the last pass-2 mms
                # so only the final scale remains on the tail chain.
                nc.vector.scalar_tensor_tensor(
                    mexp0, exp_sb, 1.0, mask0, op0=Alu.mult, op1=Alu.mult,
                    accum_out=psums01[:, 0:1],
                )
                nc.vector.scalar_tensor_tensor(
                    mexp1, exp_sb, 1.0, mask1, op0=Alu.mult, op1=Alu.mult,
                    accum_out=psums01[:, 1:2],
                )
                pst = psS.tile([BL, 1], f32, tag="s", name="pst")
                nc.tensor.matmul(pst, psums01, ones_sb[:, 0:1], start=True, stop=True)
                rinv2 = consts.tile([BL, 1], f32)
                nc.vector.reciprocal(rinv2, pst)
                emit_pass2(pend[0])
                assert p2_emitted == nf
                ctx_sb = consts.tile([BL, d], f32)
                nc.vector.tensor_scalar_mul(ctx_sb, cps, rinv2)
                nc.gpsimd.dma_start(out=out_d[:, :], in_=ctx_sb)

    nc.compile()
    return nc


def _get_nc(nt, stage="all"):
    key = (nt, stage)
    if key not in _NC_CACHE:
        _NC_CACHE[key] = _build_program(nt, stage)
    return _NC_CACHE[key]


def _plan(lengths):
    """Pair batches (longest padded length with shortest) so every core's
    two batches need the same, minimal number of 512-wide s-tiles."""
    l = np.asarray(lengths, dtype=np.int64)
    c = (np.clip(l, 1, S) + ST - 1) // ST          # tiles per batch, >= 1
    order = np.argsort(-c, kind="stable")
    pairs = [(int(order[i]), int(order[B - 1 - i])) for i in range(NCORES)]
    nt = int(max(c[a] + c[b] for a, b in pairs))
    return pairs, c, nt


def _make_in_maps(encoder_outputs, hidden, lengths, W, b, v):
    import ml_dtypes

    bf16 = ml_dtypes.bfloat16
    enc = np.asarray(encoder_outputs, dtype=np.float32)
    hid = np.asarray(hidden, dtype=np.float32)
    len_ = np.asarray(lengths, dtype=np.int64)
    Wn = np.asarray(W, dtype=np.float32)
    bn = np.asarray(b, dtype=np.float32)
    vn = np.asarray(v, dtype=np.float32)

    pairs, c, nt = _plan(len_)
    nf = nt * NPT

    # per-ec stripes, partition-major within each stripe:
    # [ec, p, kc, q] = W.T[kc*128 + p, ec*128 + q]
    weTs = np.ascontiguousarray(
        Wn[:, D:].T.reshape(DC, 128, DC, 128).transpose(2, 1, 0, 3).astype(bf16)
    )
    whTs = np.ascontiguousarray(
        Wn[:, :D].T.reshape(DC, 128, DC, 128).transpose(2, 1, 0, 3).astype(bf16)
    )
    bcol = np.ascontiguousarray(bn.reshape(DC, 128).T)
    vcol = np.ascontiguousarray(vn.reshape(DC, 128).T)

    in_maps = []
    for a, b_ in pairs:
        na, nb = int(c[a]), int(c[b_])
        packed = np.zeros((nt * ST, D), dtype=np.float32)
        packed[:na * ST] = enc[a, :na * ST]
        packed[na * ST:(na + nb) * ST] = enc[b_, :nb * ST]
        packed = packed.astype(bf16)
        encTf = np.ascontiguousarray(
            packed.reshape(nt, ST, DC, 128).transpose(0, 3, 2, 1)
        )
        encf = packed.reshape(nf, 128, D)

        hidf = np.zeros((D, nt), dtype=np.float32)
        hidf[:, :na] = hid[a][:, None]
        hidf[:, na:na + nb] = hid[b_][:, None]
        hidf = np.ascontiguousarray(
            hidf.astype(bf16).reshape(DC, 128, nt).transpose(1, 0, 2)
        )

        posf = np.full((128, nf), 1.0e9, dtype=np.float32)
        lenmap = np.zeros((128, nf), dtype=np.float32)
        own0 = np.zeros((128, nf), dtype=np.float32)
        p = np.arange(128, dtype=np.float32)
        for f in range(nf):
            t = f // NPT
            if t < na:
                posf[:, f] = f * 128 + p
                lenmap[:, f] = float(len_[a])
                own0[:, f] = 1.0
            elif t < na + nb:
                posf[:, f] = (f - na * NPT) * 128 + p
                lenmap[:, f] = float(len_[b_])

        in_maps.append(
            dict(
                encTf=encTf, encf=np.ascontiguousarray(encf),
                weTs=weTs, whTs=whTs, hidf=hidf,
                bcol=bcol, vcol=vcol,
                posf=posf, lenmap=lenmap, own0=own0,
            )
        )
    return in_maps, pairs, nt


def run(inputs, trace=False, stage="all"):
    """Run on 8 NeuronCores; returns (output [B,1,D], BassKernelResults)."""
    from concourse.bass_utils import run_bass_kernel_spmd

    in_maps, pairs, nt = _make_in_maps(**inputs)
    nc = _get_nc(nt, stage)
    r = run_bass_kernel_spmd(
        nc, in_maps, core_ids=list(range(NCORES)), trace=trace
    )
    if stage != "all":
        out = np.stack(
            [np.asarray(r.results[i]["ctx_out"]) for i in range(NCORES)], axis=0
        )
        return out, r, pairs
    out = np.empty((B, 1, D), dtype=np.float32)
    for i, (a, b_) in enumerate(pairs):
        ctx = np.asarray(r.results[i]["ctx_out"])
        out[a, 0] = ctx[0]
        out[b_, 0] = ctx[1]
    return out, r


def kernel(encoder_outputs, hidden, lengths, W, b, v):
    out, _ = run(
        dict(
            encoder_outputs=encoder_outputs,
            hidden=hidden,
            lengths=lengths,
            W=W,
            b=b,
            v=v,
        )
    )
    return out
